# revision 34
# baseline (speedup 1.0000x reference)
"""GQA (B=2, S=2048, d_model=2048, 16 Q heads / 4 KV groups) + output projection.

Sharding: 8 cores, core c <-> (b = c//4, g = c%4). Each core computes full
attention for the 4 query heads of KV group g of batch b, then multiplies its
512-feature slice of the concatenated head outputs with the matching 512 rows
of Wc^T, producing a partial [S, d_model] projection (bf16). Host sums the 4
partials per batch element and adds the bias.

On-core layout: everything transposed, all matmul operands bf16 (validated
rel-err 3.8e-3 in numpy sim vs the 2e-2 budget; bf16 halves DMA + weight-load
traffic vs the earlier f32r version at the same 1 cycle/row stream rate).
  scoresT[t, s] = kT.T @ qT            (lhsT = kT tile [d,128t], rhs = qT [d,512s])
  expT = exp(scoresT / sqrt(128))      (ACT, fused scale, bf16 out)
  sums  = partition_all_reduce(incremental pair-tree(expT))
                                       (DVE bf16 adds streamed as exp chunks
                                        land, so the Pool all-reduce issues
                                        ~3us earlier than a post-hoc tree)
  uT[hd, s]   = v.T @ expT             (PE, bf16, accumulated over 16 t)
  attnT = uT * recip(sums)             (DVE, emitted mid-iteration k+1: late
                                        enough that the Pool all-reduce is
                                        done (no DVE head-of-line stall),
                                        early enough to unblock proj starts)
  out[s, o]   = attnT.T @ wT           (PE bf16, contraction over 512 features;
                                        bias added on host)

PE pstate discipline: the TRN2 tensor engine drops to half clock for ~3us
after ANY idle gap. Two structural idle sources are plugged with dummy
matmuls on a memset scratch tile: (1) the ~6us engine-boot + first-DMA-ring
latency before kT/qT land, (2) iterations 0-4 where ACT (exp, 8.3us/combo)
outpaces PE (QK+PV only, 6.8us/combo) because no attnT exists yet for proj
work. Dummies keep the PE stream gap-free so real matmuls run at 2.4GHz.

Scheduling: per-iteration k the PE stream interleaves, per 2-t-tile step,
QK(k) pairs + PV(k-1) pairs + 2 proj matmuls of the slice (jp=(k-5)//4,
st=(k-5)%4). Output DMA per orow half right after its copies complete.
"""

import math
import sys

sys.path.insert(0, "/opt/trn_rl_repo")

import ml_dtypes
import numpy as np

import concourse.bacc as bacc
import concourse.bass as bass
import concourse.bass_isa as bass_isa
import concourse.mybir as mybir
import concourse.tile as tile
from concourse.bass import ds, ts
from concourse.bass_utils import run_bass_kernel_spmd

F32 = mybir.dt.float32
BF16 = mybir.dt.bfloat16

B = 2
S = 2048
D_MODEL = 2048
N_GROUPS = 4
HEADS_PER_GROUP = 4
HEAD_DIM = 128
P = 128
NT = S // P          # 16 t tiles
SCALE = 1.0 / math.sqrt(HEAD_DIM)

_COMPILED = None

# Schedule knobs (tuned via TimelineSim sweep + HW verification):
#   startup_dummies: PE pstate-warming matmuls before the first real QK
#   filler: extra per-cc dummy matmuls in ACT-bound iterations (0-4: no proj
#           work exists yet so PE trails ACT; 16: short bridge iteration)
#   pv_lag: chunks the PV stream trails the exp stream by (PV(k-1) finishes
#           at cc == pv_lag - 1 of iteration k)
#   at_cc:  cc step at which combo k-1's normalize (recip+mult) is emitted
#   qk_pos: "first" or "last" within each cc step's PE emission block
SCHEDULE = dict(
    startup_dummies=12,
    filler={0: 3, 1: 2, 2: 2, 3: 2, 4: 2, 16: 1},
    pv_lag=2,
    at_cc=4,
)
# cc-steps the QK+exp streams lead the PV/proj/tree streams by
QK_LEAD = 2


def _build():
    nc = bacc.Bacc(None, target_bir_lowering=False)

    qT_d = nc.dram_tensor("qT", [P, HEADS_PER_GROUP, S], BF16, kind="ExternalInput")
    kT_d = nc.dram_tensor("kT", [P, S], BF16, kind="ExternalInput")
    v_d = nc.dram_tensor("v", [S, P], BF16, kind="ExternalInput")
    wT_d = nc.dram_tensor("wT", [HEADS_PER_GROUP * P, D_MODEL], BF16, kind="ExternalInput")
    out_d = nc.dram_tensor("out", [S, D_MODEL], BF16, kind="ExternalOutput")

    Exp = mybir.ActivationFunctionType.Exp
    add = mybir.AluOpType.add
    mult = mybir.AluOpType.mult

    n_combos = 16

    with tile.TileContext(nc) as tc:
        with (
            tc.tile_pool(name="const", bufs=1) as const_pool,
            tc.tile_pool(name="qt", bufs=3) as qt_pool,
            tc.tile_pool(name="expT", bufs=3) as expT_pool,
            tc.tile_pool(name="ta", bufs=8) as ta_pool,
            tc.tile_pool(name="tb", bufs=6) as tb_pool,
            tc.tile_pool(name="acc", bufs=2) as acc_pool,
            tc.tile_pool(name="sums", bufs=2) as sums_pool,
            tc.tile_pool(name="rb", bufs=2) as rb_pool,
            tc.tile_pool(name="attnT", bufs=8) as attnT_pool,
            tc.tile_pool(name="orow", bufs=2) as orow_pool,
            tc.tile_pool(name="qk_ps", bufs=2, space="PSUM") as qk_psum,
            tc.tile_pool(name="pv_ps", bufs=2, space="PSUM") as pv_psum,
            tc.tile_pool(name="po_ps", bufs=2, space="PSUM") as po_psum,
        ):
            # startup: spread the first-wave DMAs over 4 engine queues so the
            # rings warm in parallel; the first QK needs only kT chunk 0 + qT0.
            scratch = const_pool.tile([P, 512], BF16, tag="scratch")
            nc.gpsimd.memset(scratch[:], 0.0)

            kT_chunks = []
            for c in range(4):
                kc = const_pool.tile([P, 512], BF16, tag=f"kT{c}")
                kT_chunks.append(kc)
            qt0 = qt_pool.tile([P, 512], BF16, tag="qT")
            nc.sync.dma_start(qt0[:], qT_d[:, 0, ts(0, 512)])
            nc.sync.dma_start(kT_chunks[0][:], kT_d[:, ts(0, 512)])
            nc.scalar.dma_start(kT_chunks[1][:], kT_d[:, ts(1, 512)])
            qt1 = qt_pool.tile([P, 512], BF16, tag="qT")
            nc.scalar.dma_start(qt1[:], qT_d[:, 1, ts(0, 512)])
            nc.sync.dma_start(kT_chunks[2][:], kT_d[:, ts(2, 512)])
            nc.gpsimd.dma_start(kT_chunks[3][:], kT_d[:, ts(3, 512)])
            # v (256KB) then wT (2MB) on the gpsimd queue -- first use of wT
            # is the first proj slice ~50us in.
            v_sb = const_pool.tile([P, NT, P], BF16, tag="v")
            nc.gpsimd.dma_start(v_sb[:], v_d.rearrange("(n p) d -> p n d", p=P))
            wT_sb = const_pool.tile([P, HEADS_PER_GROUP, D_MODEL], BF16, tag="wT")
            nc.gpsimd.dma_start(wT_sb[:], wT_d.rearrange("(n p) o -> p n o", p=P))

            # pre-warm the PE pstate while the first DMAs land
            warm_ps = po_psum.tile([P, 512], F32, tag="po")
            for _ in range(SCHEDULE["startup_dummies"]):
                nc.tensor.matmul(
                    warm_ps[:], scratch[:, ts(0, P)], scratch[:],
                    start=True, stop=True,
                )

            def dummy():
                wp = po_psum.tile([P, 512], F32, tag="po")
                nc.tensor.matmul(
                    wp[:], scratch[:, ts(0, P)], scratch[:],
                    start=True, stop=True,
                )

            qts = {0: qt0, 1: qt1}
            ets = {}      # k -> exp tile [P, 16, 512] bf16
            attnT = {}
            pv_tiles = {}
            sums_tiles = {}

            # lead primer: combo 0's first QK_LEAD chunks + exps, so that
            # iteration 0's cc stream starts already QK_LEAD steps ahead
            et0 = expT_pool.tile([P, NT, 512], BF16, tag="expT")
            ets[0] = et0
            for c0 in range(QK_LEAD):
                ps0 = qk_psum.tile([P, 2, 512], F32, tag="qk")
                for u in range(2):
                    tt = 2 * c0 + u
                    nc.tensor.matmul(
                        ps0[:, u, :],
                        kT_chunks[tt // 4][:, ts(tt % 4, P)],
                        qts[0][:],
                        start=True, stop=True,
                    )
                nc.scalar.activation(
                    et0[:, ds(2 * c0, 2), :], ps0[:], Exp, scale=SCALE
                )

            for k in range(n_combos + 5):
                # prefetch qT two iterations ahead on the gpsimd queue (only
                # SP/Activation/gpsimd can ring DMA doorbells; gpsimd runs one
                # all-reduce per iteration and has plenty of slack)
                kq = k + 2
                if kq < n_combos and kq not in qts:
                    jq, hq = divmod(kq, HEADS_PER_GROUP)
                    qt = qt_pool.tile([P, 512], BF16, tag="qT")
                    nc.gpsimd.dma_start(qt[:], qT_d[:, hq, ts(jq, 512)])
                    qts[kq] = qt

                do_qk = k < n_combos
                # proj slice: group jp, row-block st, fed by combos of group jp+1
                do_proj = 5 <= k <= n_combos + 4
                if do_proj:
                    jp, stp = divmod(k - 5, 4)
                    jp_orow = orow_pool.tile([P, D_MODEL], BF16, tag="orow")

                if do_qk:
                    a_tiles = []
                    p_tiles = []

                def emit_at(km1):
                    rbt = rb_pool.tile([P, 512], F32, tag="rb")
                    nc.vector.reciprocal_approx_fast(rbt[:], sums_tiles[km1][:])
                    at = attnT_pool.tile([P, 512], BF16, tag="attnT")
                    nc.vector.tensor_tensor(
                        at[:], pv_tiles[km1][:], rbt[:], mult
                    )
                    attnT[km1] = at

                pv_lag = SCHEDULE["pv_lag"]
                for cc in range(8):
                    # The QK+exp streams run QK_LEAD cc-steps ahead of the
                    # PV/proj/tree streams: combo k+1's first chunks are
                    # emitted at the tail of iteration k. This keeps the ACT
                    # exp stream continuous across iteration boundaries (no
                    # QK(cc0) -> exp serialization) and makes every exp chunk
                    # ready before its DVE/PV consumers, collapsing the
                    # embedded waits that were saturating the DVE queue.
                    qk_k, qk_c = (k, cc + QK_LEAD) if cc < 8 - QK_LEAD else (
                        k + 1, cc - (8 - QK_LEAD))
                    do_qk_here = qk_k < n_combos

                    if do_qk_here:
                        if qk_k not in ets:
                            et_new = expT_pool.tile([P, NT, 512], BF16, tag="expT")
                            ets[qk_k] = et_new
                        ps = qk_psum.tile([P, 2, 512], F32, tag="qk")
                        for u in range(2):
                            tt = 2 * qk_c + u
                            nc.tensor.matmul(
                                ps[:, u, :],
                                kT_chunks[tt // 4][:, ts(tt % 4, P)],
                                qts[qk_k][:],
                                start=True, stop=True,
                            )
                    # PV stream trails the exp stream by pv_lag chunks: combo
                    # k's head tiles run this iteration right behind exp(k),
                    # the last 2*pv_lag tiles land at cc 0..pv_lag-1 of the
                    # next iteration, so PV(k-1) is complete well before the
                    # normalize at at_cc reads its accumulation.
                    if cc < pv_lag:
                        m = k - 1
                        tts_pv = (16 - 2 * pv_lag + 2 * cc, 17 - 2 * pv_lag + 2 * cc)
                    else:
                        m = k
                        tts_pv = (2 * (cc - pv_lag), 2 * (cc - pv_lag) + 1)
                    if 0 <= m < n_combos:
                        if m == k and m not in pv_tiles:
                            pv_ps = pv_psum.tile([P, 512], F32, tag="pv")
                            pv_tiles[k] = pv_ps
                        for tt in tts_pv:
                            nc.tensor.matmul(
                                pv_tiles[m][:],
                                v_sb[:, tt, :],
                                ets[m][:, tt, :],
                                start=(tt == 0), stop=(tt == NT - 1),
                            )
                    pending_copy = None
                    if do_proj:
                        ob, uh = divmod(cc, 2)
                        if uh == 0:
                            po = po_psum.tile([P, 512], F32, tag="po")
                        for h in (0, 1) if uh == 0 else (2, 3):
                            nc.tensor.matmul(
                                po[:],
                                attnT[4 * jp + h][:, ts(stp, P)],
                                wT_sb[:, h, ts(ob, 512)],
                                start=(h == 0), stop=(h == HEADS_PER_GROUP - 1),
                            )
                        if uh == 1:
                            pending_copy = (po, ob)
                    if k in SCHEDULE["filler"]:
                        for _ in range(SCHEDULE["filler"][k]):
                            dummy()
                    if do_qk_here:
                        nc.scalar.activation(
                            ets[qk_k][:, ds(2 * qk_c, 2), :], ps[:], Exp,
                            scale=SCALE,
                        )
                    # proj copies: ob0-2 on DVE in po-recycle-deadline order
                    # (ob2 ahead of its cc's tree work), ob3 on ACT where the
                    # led exp stream leaves ~1.4us of slack per iteration.
                    def emit_copy():
                        po_c, ob = pending_copy
                        if ob == 3:
                            nc.scalar.copy(jp_orow[:, ts(ob, 512)], po_c[:])
                        else:
                            nc.vector.tensor_copy(jp_orow[:, ts(ob, 512)], po_c[:])
                        if ob == 1 or ob == 3:
                            # stream each orow half out as soon as its
                            # copies land (halves the end-of-kernel drain)
                            half = (ob - 1) // 2
                            nc.sync.dma_start(
                                out_d[ds(jp * 512 + stp * P, P),
                                      ts(half, 1024)],
                                jp_orow[:, ts(half, 1024)],
                            )

                    if pending_copy is not None and cc >= 5:
                        emit_copy()          # ob2 @ cc5, ob3 @ cc7: ahead of tree
                        pending_copy = None
                    if do_qk:
                        if cc % 2 == 1:
                            # incremental softmax-denominator: pair-add the 4
                            # t-tiles that just finished exp'ing, then fold
                            # into a running partial so only a_3 + 2 folds
                            # remain after exp(cc7) -- the shorter this tail,
                            # the sooner the Pool all-reduce can issue.
                            i = (cc - 1) // 2
                            a = ta_pool.tile([P, 2, 512], BF16, tag="ta")
                            nc.vector.tensor_tensor(
                                a[:], ets[k][:, ds(4 * i, 2), :],
                                ets[k][:, ds(4 * i + 2, 2), :], add,
                            )
                            a_tiles.append(a)
                            if i >= 1:
                                pt = tb_pool.tile([P, 2, 512], BF16, tag="tb")
                                nc.vector.tensor_tensor(
                                    pt[:],
                                    (a_tiles[0] if i == 1 else p_tiles[-1])[:],
                                    a[:], add,
                                )
                                p_tiles.append(pt)
                    if pending_copy is not None:
                        emit_copy()          # ob0 @ cc1, ob1 @ cc3: after tree adds
                        pending_copy = None
                    if cc == 2 and k == n_combos:
                        # last combo's normalize: no epilogue at k=16, and
                        # PV(15) stops at cc1 of this iteration
                        emit_at(k - 1)

                if do_qk:
                    acc = acc_pool.tile([P, 512], F32, tag="acc")
                    nc.vector.tensor_tensor(
                        acc[:], p_tiles[-1][:, 0, :], p_tiles[-1][:, 1, :], add
                    )
                    sums_bc = sums_pool.tile([P, 512], F32, tag="sums")
                    nc.gpsimd.partition_all_reduce(
                        sums_bc[:], acc[:], channels=P,
                        reduce_op=bass_isa.ReduceOp.add,
                    )
                    sums_tiles[k] = sums_bc
                    if k >= 1:
                        # normalize combo k-1 at the very TAIL of this
                        # iteration's DVE queue, after acc(k) and the
                        # all-reduce issue: rb's wait on all-reduce(k-1) can
                        # then never bubble the proj copies or delay acc --
                        # the all-reduce rides the iteration boundary instead
                        # of sitting inside the DVE critical loop. attnT(k-1)
                        # still lands an iteration before its proj group.
                        emit_at(k - 1)

    nc.compile()
    return nc


def _get_nc():
    global _COMPILED
    if _COMPILED is None:
        _COMPILED = _build()
    return _COMPILED


def _shard_inputs(q, k, v, Wc):
    bf = ml_dtypes.bfloat16
    in_maps = []
    for c in range(8):
        b, g = divmod(c, 4)
        qT = np.ascontiguousarray(
            q[b][:, g * 512:(g + 1) * 512].reshape(S, HEADS_PER_GROUP, P).transpose(2, 1, 0)
        ).astype(bf)
        kT = np.ascontiguousarray(k[b][:, g * P:(g + 1) * P].T).astype(bf)
        vv = np.ascontiguousarray(v[b][:, g * P:(g + 1) * P]).astype(bf)
        wT = np.ascontiguousarray(Wc[:, g * 512:(g + 1) * 512].T).astype(bf)
        in_maps.append({"qT": qT, "kT": kT, "v": vv, "wT": wT})
    return in_maps


def _run(inputs, trace=False):
    q = np.asarray(inputs["q"], dtype=np.float32)
    k = np.asarray(inputs["k"], dtype=np.float32)
    v = np.asarray(inputs["v"], dtype=np.float32)
    Wc = np.asarray(inputs["Wc"], dtype=np.float32)
    bc = np.asarray(inputs["bc"], dtype=np.float32)

    nc = _get_nc()
    in_maps = _shard_inputs(q, k, v, Wc)
    res = run_bass_kernel_spmd(nc, in_maps, list(range(8)), trace=trace)

    out = np.empty((B, S, D_MODEL), dtype=np.float32)
    for b in range(B):
        acc = res.results[4 * b]["out"].astype(np.float32)
        for g in range(1, 4):
            acc = acc + res.results[4 * b + g]["out"].astype(np.float32)
        acc += bc.reshape(1, D_MODEL)
        out[b] = acc
    return out, res


def kernel(**inputs):
    out, _ = _run(inputs, trace=False)
    return out
